# revision 14
# baseline (speedup 1.0000x reference)
"""Trainium2 Bass kernel for BeamDelay2AntFreq_3D (nn_BeamDelay2AntFreq_3D).

Math: x [n_b, n_c, h=8, v=4, pol=2, t=48] complex (given as separate
real/imag float32 tensors).
  out[b, c, (h',v',p), f] = sum_{h,v,t} Mh[h',h] Mv[v',v] Mt[f,t] x[b,c,h,v,p,t]
with Mh/Mv = ortho ifft(ifftshift(.)) DFT matrices, Mt = ortho fft(fftshift(.)).
Output complex64 [n_b, n_c, 64, 48].

Sharding: pure data parallel — batch dim (256) split over 8 cores; per core
BC = 1024 independent blocks of [64, 48] complex.

Device pipeline (per 2 blocks):
  Stage 1 (one matmul; fuses the hv-transform with the o<->t transpose):
    lhsT = X tile [128 = (re hvp | im hvp), 96 = (bc2, t)]   f16 data
    rhs  = W1 [128, 128 = (ri, o)]                            f16 const
    psum Y2 [96 = (bc2, t), 128 = (ri, o)]  (fp32 accum)
  Stage 2 (two accumulating matmuls; t-transform, data as stationary):
    lhsT = Y2[:, ri*64:(ri+1)*64] [96, 64 = o] (f16 SBUF copy of psum)
    rhs  = W2A/W2B [96 = (bc2, t), 192 = (bc2, f, ri)]        f16 consts
    psum O [64 = o, 192]; tile_position (0,0)/(0,64) packs two groups
    into 128 partitions.
  PSUM->SBUF copies: ScalarE for stage 1, VectorE for the output.

DMA layouts are host-side reformatted for line-rate DMA (3-6 KB contiguous
runs per partition):
  xin  [128 = (ri, hvp), BC, 48] f16
  outp [128 = (half, o), BC/2, 96] f32, complex-interleaved pairs; host
  unpacks to [BC, 64, 48] complex64.
"""

import numpy as np

import concourse.mybir as mybir
from concourse import bacc
from concourse.tile import TileContext
from concourse.bass_utils import run_bass_kernel_spmd

N_B, N_C, N_H, N_V, N_P, N_T = 256, 32, 8, 4, 2, 48
N_CORES = 8
HVP = N_H * N_V * N_P  # 64
NF = N_T  # 48
BC_FULL = (N_B // N_CORES) * N_C  # 1024 blocks per core

SUP = 64   # bc per DMA super-tile
SUB = 16   # bc per compute sub-tile

W_DT = mybir.dt.float16
W_NP = np.float16


def _transform_mats():
    Mv = np.fft.ifft(np.fft.ifftshift(np.eye(N_V), axes=0), axis=0, norm="ortho")
    Mh = np.fft.ifft(np.fft.ifftshift(np.eye(N_H), axes=0), axis=0, norm="ortho")
    Mt = np.fft.fft(np.fft.fftshift(np.eye(N_T), axes=0), axis=0, norm="ortho")
    M1 = np.kron(np.kron(Mh, Mv), np.eye(N_P))  # [64, 64] complex
    return M1, Mt


def host_consts():
    """W1 [128, 128], W2A/W2B [96, 192] numpy arrays (W_NP dtype)."""
    M1, Mt = _transform_mats()
    M1r, M1i = M1.real, M1.imag
    Mtr, Mti = Mt.real, Mt.imag

    W1 = np.zeros((128, 128), np.float64)
    W1[0:64, 0:64] = M1r.T
    W1[0:64, 64:128] = M1i.T
    W1[64:128, 0:64] = -M1i.T
    W1[64:128, 64:128] = M1r.T

    A_blk = np.zeros((N_T, 2 * NF), np.float64)
    A_blk[:, 0::2] = Mtr.T
    A_blk[:, 1::2] = Mti.T
    B_blk = np.zeros((N_T, 2 * NF), np.float64)
    B_blk[:, 0::2] = -Mti.T
    B_blk[:, 1::2] = Mtr.T
    W2A = np.zeros((96, 192), np.float64)
    W2B = np.zeros((96, 192), np.float64)
    for g in range(2):
        W2A[g * 48:(g + 1) * 48, g * 96:(g + 1) * 96] = A_blk
        W2B[g * 48:(g + 1) * 48, g * 96:(g + 1) * 96] = B_blk
    return W1.astype(W_NP), W2A.astype(W_NP), W2B.astype(W_NP)


def build_nc(bc: int, num_devices: int = N_CORES):
    assert bc % SUP == 0
    nc = bacc.Bacc("TRN2", num_devices=num_devices)

    xin = nc.dram_tensor("xin", [128, bc, N_T], mybir.dt.float16,
                         kind="ExternalInput")
    w1_d = nc.dram_tensor("w1", [128, 128], W_DT, kind="ExternalInput")
    w2a_d = nc.dram_tensor("w2a", [96, 192], W_DT, kind="ExternalInput")
    w2b_d = nc.dram_tensor("w2b", [96, 192], W_DT, kind="ExternalInput")
    outp = nc.dram_tensor("outp", [128, bc // 2, 2 * NF], mybir.dt.float32,
                          kind="ExternalOutput")

    n_sub = SUP // SUB          # 4
    half_pairs = SUP // 4       # 8

    with TileContext(nc) as tc:
        with (
            tc.tile_pool(name="consts", bufs=1) as cpool,
            tc.tile_pool(name="xin_p", bufs=4) as xpool,
            tc.tile_pool(name="y2", bufs=4) as ypool,
            tc.tile_pool(name="outs", bufs=4) as opool,
            tc.tile_pool(name="ps1", bufs=2, space="PSUM") as ps1pool,
            tc.tile_pool(name="ps2", bufs=2, space="PSUM") as ps2pool,
        ):
            w1_t = cpool.tile([128, 128], W_DT, tag="w1")
            w2a_t = cpool.tile([96, 192], W_DT, tag="w2a")
            w2b_t = cpool.tile([96, 192], W_DT, tag="w2b")
            nc.sync.dma_start(w1_t[:, :], w1_d[:, :])
            nc.sync.dma_start(w2a_t[:, :], w2a_d[:, :])
            nc.sync.dma_start(w2b_t[:, :], w2b_d[:, :])

            for it in range(bc // SUP):
                b0 = it * SUP
                x_t = xpool.tile([128, SUP * N_T], mybir.dt.float16, tag="x")
                nc.sync.dma_start(
                    x_t[:, :].rearrange("p (b t) -> p b t", t=N_T),
                    xin[:, b0:b0 + SUP, :],
                )

                o_t = opool.tile([128, (SUP // 2) * 2 * NF],
                                 mybir.dt.float32, tag="o")

                nq = SUB // 4  # 4-bc blocks per sub-tile
                for s in range(n_sub):
                    # sub-tile s covers bc [b0 + s*SUB, b0 + (s+1)*SUB):
                    # 2*nq consecutive pairs starting at pair p0
                    p0 = s * SUB // 2

                    p1 = ps1pool.tile([96, (SUB // 2) * 128],
                                      mybir.dt.float32, tag="p1")
                    for g in range(SUB // 2):
                        pr = p0 + g
                        nc.tensor.matmul(
                            p1[:, g * 128:(g + 1) * 128],
                            lhsT=x_t[:, pr * 96:(pr + 1) * 96],
                            rhs=w1_t[:, :],
                            start=True, stop=True,
                        )
                    # y2 layout (qb, ri, pair, o): stage-2 lhsT slices are
                    # contiguous 128-col blocks. Two permuting copies (one
                    # per ri) keep every AP within 3 free dims.
                    y2 = ypool.tile([96, (SUB // 2) * 128], mybir.dt.float16,
                                    tag="y2")
                    p1v = p1[:, :].rearrange(
                        "k (qb pr ri o) -> k qb pr ri o",
                        qb=SUB // 4, pr=2, ri=2)
                    y2v = y2[:, :].rearrange(
                        "k (qb ri pr o) -> k qb ri pr o",
                        qb=SUB // 4, ri=2, pr=2)
                    for ri in range(2):
                        nc.scalar.copy(y2v[:, :, ri, :, :],
                                       p1v[:, :, :, ri, :])

                    # stage 2: one A/B matmul pair per 4 bc (2 y2 pairs),
                    # lhsT free = (pair, o) -> out partitions (pair, o).
                    # psum q-blocks gap-packed 2-per-bank.
                    p2 = ps2pool.tile([128, nq * 192 + (nq // 2) * 128],
                                      mybir.dt.float32, tag="p2")
                    for q in range(nq):
                        c0 = q * 192 + (q // 2) * 128
                        lhsA = y2[:, q * 256:q * 256 + 128]
                        lhsB = y2[:, q * 256 + 128:q * 256 + 256]
                        nc.tensor.matmul(
                            p2[:, c0:c0 + 192], lhsT=lhsA, rhs=w2a_t[:, :],
                            start=True, stop=False,
                        )
                        nc.tensor.matmul(
                            p2[:, c0:c0 + 192], lhsT=lhsB, rhs=w2b_t[:, :],
                            start=False, stop=True,
                        )

                    # copy the nq valid 192-col blocks (skip bank gaps)
                    nc.vector.tensor_copy(
                        o_t[:, s * nq * 192:(s + 1) * nq * 192].rearrange(
                            "p (b c) -> p b c", c=384),
                        p2[:, :].rearrange("p (b c) -> p b c", c=512)
                        [:, :, 0:384],
                    )

                nc.sync.dma_start(
                    outp[:, it * (SUP // 2):(it + 1) * (SUP // 2), :],
                    o_t[:, :].rearrange("p (b c) -> p b c", c=2 * NF),
                )
    return nc


def host_pack_inputs(x_real, x_imag):
    """f32 [256, 32, 8, 4, 2, 48] pair -> [cores, 128, BC, 48] f16."""
    xr = np.asarray(x_real, np.float16).reshape(N_CORES, BC_FULL, HVP, N_T)
    xi = np.asarray(x_imag, np.float16).reshape(N_CORES, BC_FULL, HVP, N_T)
    packed = np.empty((N_CORES, 128, BC_FULL, N_T), np.float16)
    packed[:, :64] = xr.transpose(0, 2, 1, 3)
    packed[:, 64:] = xi.transpose(0, 2, 1, 3)
    return packed


def host_unpack_output(outp, bc):
    """outp [128, bc/2, 96] f32 -> [bc, 64, 48] complex64.

    outp row = half*64 + o; col-slot = qg*2 + g; bc = qg*4 + half*2 + g."""
    v = outp.view(np.complex64).reshape(2, 64, bc // 4, 2, NF)
    # [half, o, qg, g, f] -> [qg, half, g, o, f]
    return np.ascontiguousarray(v.transpose(2, 0, 3, 1, 4)).reshape(
        bc, HVP, NF)


def reference_numpy(x_real, x_imag):
    """Pure-numpy reference; x_* [bc, 64, 48] -> [bc, 64, 48] complex64."""
    M1, Mt = _transform_mats()
    x = x_real.astype(np.complex64) + 1j * x_imag.astype(np.complex64)
    y = np.einsum("oi,bit->bot", M1, x)
    return np.einsum("ft,bot->bof", Mt, y).astype(np.complex64)


_NC_CACHE = {}


def _get_nc(bc):
    if bc not in _NC_CACHE:
        nc = build_nc(bc)
        nc.compile()
        _NC_CACHE[bc] = nc
    return _NC_CACHE[bc]


def _run(x_real: np.ndarray, x_imag: np.ndarray, **spmd_kwargs):
    assert x_real.shape == (N_B, N_C, N_H, N_V, N_P, N_T)
    packed = host_pack_inputs(x_real, x_imag)
    W1, W2A, W2B = host_consts()

    nc = _get_nc(BC_FULL)
    in_maps = [
        {"xin": packed[i], "w1": W1, "w2a": W2A, "w2b": W2B}
        for i in range(N_CORES)
    ]
    res = run_bass_kernel_spmd(nc, in_maps, core_ids=list(range(N_CORES)),
                               **spmd_kwargs)
    outs = [
        host_unpack_output(r["outp"], BC_FULL).reshape(
            N_B // N_CORES, N_C, HVP, NF)
        for r in res.results
    ]
    return np.concatenate(outs, axis=0), res


def kernel(x_real: np.ndarray, x_imag: np.ndarray) -> np.ndarray:
    return _run(x_real, x_imag)[0]


# revision 16
# speedup vs baseline: 1.0564x; 1.0564x over previous
"""Trainium2 Bass kernel for BeamDelay2AntFreq_3D (nn_BeamDelay2AntFreq_3D).

Math: x [n_b, n_c, h=8, v=4, pol=2, t=48] complex (given as separate
real/imag float32 tensors).
  out[b, c, (h',v',p), f] = sum_{h,v,t} Mh[h',h] Mv[v',v] Mt[f,t] x[b,c,h,v,p,t]
with Mh/Mv = ortho ifft(ifftshift(.)) DFT matrices, Mt = ortho fft(fftshift(.)).
Output complex64 [n_b, n_c, 64, 48].

Sharding: pure data parallel — batch dim (256) split over 8 cores; per core
BC = 1024 independent blocks of [64, 48] complex.

Device pipeline (per 2 blocks):
  Stage 1 (one matmul; fuses the hv-transform with the o<->t transpose):
    lhsT = X tile [128 = (re hvp | im hvp), 96 = (bc2, t)]   f16 data
    rhs  = W1 [128, 128 = (ri, o)]                            f16 const
    psum Y2 [96 = (bc2, t), 128 = (ri, o)]  (fp32 accum)
  Stage 2 (two accumulating matmuls; t-transform, data as stationary):
    lhsT = Y2[:, ri*64:(ri+1)*64] [96, 64 = o] (f16 SBUF copy of psum)
    rhs  = W2A/W2B [96 = (bc2, t), 192 = (bc2, f, ri)]        f16 consts
    psum O [64 = o, 192]; tile_position (0,0)/(0,64) packs two groups
    into 128 partitions.
  PSUM->SBUF copies: ScalarE for stage 1, VectorE for the output.

DMA layouts are host-side reformatted for line-rate DMA (3-6 KB contiguous
runs per partition):
  xin  [128 = (ri, hvp), BC, 48] f16
  outp [128 = (half, o), BC/2, 96] f32, complex-interleaved pairs; host
  unpacks to [BC, 64, 48] complex64.
"""

import numpy as np

import concourse.mybir as mybir
from concourse import bacc
from concourse.tile import TileContext
from concourse.bass_utils import run_bass_kernel_spmd

N_B, N_C, N_H, N_V, N_P, N_T = 256, 32, 8, 4, 2, 48
N_CORES = 8
HVP = N_H * N_V * N_P  # 64
NF = N_T  # 48
BC_FULL = (N_B // N_CORES) * N_C  # 1024 blocks per core

SUP = 64   # bc per DMA super-tile
SUB = 16   # bc per compute sub-tile

W_DT = mybir.dt.float16
W_NP = np.float16


def _transform_mats():
    Mv = np.fft.ifft(np.fft.ifftshift(np.eye(N_V), axes=0), axis=0, norm="ortho")
    Mh = np.fft.ifft(np.fft.ifftshift(np.eye(N_H), axes=0), axis=0, norm="ortho")
    Mt = np.fft.fft(np.fft.fftshift(np.eye(N_T), axes=0), axis=0, norm="ortho")
    M1 = np.kron(np.kron(Mh, Mv), np.eye(N_P))  # [64, 64] complex
    return M1, Mt


def host_consts():
    """W1 [128, 128], W2A/W2B [96, 192] numpy arrays (W_NP dtype)."""
    M1, Mt = _transform_mats()
    M1r, M1i = M1.real, M1.imag
    Mtr, Mti = Mt.real, Mt.imag

    W1 = np.zeros((128, 128), np.float64)
    W1[0:64, 0:64] = M1r.T
    W1[0:64, 64:128] = M1i.T
    W1[64:128, 0:64] = -M1i.T
    W1[64:128, 64:128] = M1r.T

    A_blk = np.zeros((N_T, 2 * NF), np.float64)
    A_blk[:, 0::2] = Mtr.T
    A_blk[:, 1::2] = Mti.T
    B_blk = np.zeros((N_T, 2 * NF), np.float64)
    B_blk[:, 0::2] = -Mti.T
    B_blk[:, 1::2] = Mtr.T
    W2A = np.zeros((96, 192), np.float64)
    W2B = np.zeros((96, 192), np.float64)
    for g in range(2):
        W2A[g * 48:(g + 1) * 48, g * 96:(g + 1) * 96] = A_blk
        W2B[g * 48:(g + 1) * 48, g * 96:(g + 1) * 96] = B_blk
    return W1.astype(W_NP), W2A.astype(W_NP), W2B.astype(W_NP)


def build_nc(bc: int, num_devices: int = N_CORES):
    assert bc % SUP == 0
    nc = bacc.Bacc("TRN2", num_devices=num_devices)

    xin = nc.dram_tensor("xin", [128, bc, N_T], mybir.dt.float16,
                         kind="ExternalInput")
    w1_d = nc.dram_tensor("w1", [128, 128], W_DT, kind="ExternalInput")
    w2a_d = nc.dram_tensor("w2a", [96, 192], W_DT, kind="ExternalInput")
    w2b_d = nc.dram_tensor("w2b", [96, 192], W_DT, kind="ExternalInput")
    outp = nc.dram_tensor("outp", [128, bc // 2, 2 * NF], mybir.dt.float32,
                          kind="ExternalOutput")

    n_sub = SUP // SUB          # 4
    half_pairs = SUP // 4       # 8

    with TileContext(nc) as tc:
        with (
            tc.tile_pool(name="consts", bufs=1) as cpool,
            tc.tile_pool(name="xin_p", bufs=4) as xpool,
            tc.tile_pool(name="y2", bufs=4) as ypool,
            tc.tile_pool(name="outs", bufs=4) as opool,
            tc.tile_pool(name="ps1", bufs=3, space="PSUM") as ps1pool,
            tc.tile_pool(name="ps2", bufs=2, space="PSUM") as ps2pool,
        ):
            w1_t = cpool.tile([128, 128], W_DT, tag="w1")
            w2a_t = cpool.tile([96, 192], W_DT, tag="w2a")
            w2b_t = cpool.tile([96, 192], W_DT, tag="w2b")
            nc.sync.dma_start(w1_t[:, :], w1_d[:, :])
            nc.sync.dma_start(w2a_t[:, :], w2a_d[:, :])
            nc.sync.dma_start(w2b_t[:, :], w2b_d[:, :])

            for it in range(bc // SUP):
                b0 = it * SUP
                x_t = xpool.tile([128, SUP * N_T], mybir.dt.float16, tag="x")
                nc.sync.dma_start(
                    x_t[:, :].rearrange("p (b t) -> p b t", t=N_T),
                    xin[:, b0:b0 + SUP, :],
                )

                o_t = opool.tile([128, (SUP // 2) * 2 * NF],
                                 mybir.dt.float32, tag="o")

                nq = SUB // 4  # 4-bc blocks per sub-tile
                for s in range(n_sub):
                    # sub-tile s covers bc [b0 + s*SUB, b0 + (s+1)*SUB):
                    # 2*nq consecutive pairs starting at pair p0
                    p0 = s * SUB // 2

                    p1 = ps1pool.tile([96, (SUB // 2) * 128],
                                      mybir.dt.float32, tag="p1")
                    for g in range(SUB // 2):
                        pr = p0 + g
                        nc.tensor.matmul(
                            p1[:, g * 128:(g + 1) * 128],
                            lhsT=x_t[:, pr * 96:(pr + 1) * 96],
                            rhs=w1_t[:, :],
                            start=True, stop=True,
                        )
                    # y2 layout (qb, ri, pair, o): stage-2 lhsT slices are
                    # contiguous 128-col blocks. Two permuting copies (one
                    # per ri) keep every AP within 3 free dims.
                    y2 = ypool.tile([96, (SUB // 2) * 128], mybir.dt.float16,
                                    tag="y2")
                    p1v = p1[:, :].rearrange(
                        "k (qb pr ri o) -> k qb pr ri o",
                        qb=SUB // 4, pr=2, ri=2)
                    y2v = y2[:, :].rearrange(
                        "k (qb ri pr o) -> k qb ri pr o",
                        qb=SUB // 4, ri=2, pr=2)
                    for ri in range(2):
                        if s % 2 == 0:
                            nc.scalar.copy(y2v[:, :, ri, :, :],
                                           p1v[:, :, :, ri, :])
                        else:
                            nc.vector.tensor_copy(y2v[:, :, ri, :, :],
                                                  p1v[:, :, :, ri, :])

                    # stage 2: one A/B matmul pair per 4 bc (2 y2 pairs),
                    # lhsT free = (pair, o) -> out partitions (pair, o).
                    # Two 1-bank psum tiles per sub-tile, 2 q-blocks each.
                    for h in range(nq // 2):
                        p2 = ps2pool.tile([128, 384], mybir.dt.float32,
                                          tag="p2")
                        for qq in range(2):
                            q = 2 * h + qq
                            c0 = qq * 192
                            lhsA = y2[:, q * 256:q * 256 + 128]
                            lhsB = y2[:, q * 256 + 128:q * 256 + 256]
                            nc.tensor.matmul(
                                p2[:, c0:c0 + 192], lhsT=lhsA,
                                rhs=w2a_t[:, :], start=True, stop=False,
                            )
                            nc.tensor.matmul(
                                p2[:, c0:c0 + 192], lhsT=lhsB,
                                rhs=w2b_t[:, :], start=False, stop=True,
                            )
                        o_slice = o_t[:, (s * nq + 2 * h) * 192:
                                      (s * nq + 2 * h + 2) * 192]
                        if (s + h) % 2 == 0:
                            nc.vector.tensor_copy(o_slice, p2[:, :])
                        else:
                            nc.scalar.copy(o_slice, p2[:, :])

                nc.sync.dma_start(
                    outp[:, it * (SUP // 2):(it + 1) * (SUP // 2), :],
                    o_t[:, :].rearrange("p (b c) -> p b c", c=2 * NF),
                )
    return nc


def host_pack_inputs(x_real, x_imag):
    """f32 [256, 32, 8, 4, 2, 48] pair -> [cores, 128, BC, 48] f16."""
    xr = np.asarray(x_real, np.float16).reshape(N_CORES, BC_FULL, HVP, N_T)
    xi = np.asarray(x_imag, np.float16).reshape(N_CORES, BC_FULL, HVP, N_T)
    packed = np.empty((N_CORES, 128, BC_FULL, N_T), np.float16)
    packed[:, :64] = xr.transpose(0, 2, 1, 3)
    packed[:, 64:] = xi.transpose(0, 2, 1, 3)
    return packed


def host_unpack_output(outp, bc):
    """outp [128, bc/2, 96] f32 -> [bc, 64, 48] complex64.

    outp row = half*64 + o; col-slot = qg*2 + g; bc = qg*4 + half*2 + g."""
    v = outp.view(np.complex64).reshape(2, 64, bc // 4, 2, NF)
    # [half, o, qg, g, f] -> [qg, half, g, o, f]
    return np.ascontiguousarray(v.transpose(2, 0, 3, 1, 4)).reshape(
        bc, HVP, NF)


def reference_numpy(x_real, x_imag):
    """Pure-numpy reference; x_* [bc, 64, 48] -> [bc, 64, 48] complex64."""
    M1, Mt = _transform_mats()
    x = x_real.astype(np.complex64) + 1j * x_imag.astype(np.complex64)
    y = np.einsum("oi,bit->bot", M1, x)
    return np.einsum("ft,bot->bof", Mt, y).astype(np.complex64)


_NC_CACHE = {}


def _get_nc(bc):
    if bc not in _NC_CACHE:
        nc = build_nc(bc)
        nc.compile()
        _NC_CACHE[bc] = nc
    return _NC_CACHE[bc]


def _run(x_real: np.ndarray, x_imag: np.ndarray, **spmd_kwargs):
    assert x_real.shape == (N_B, N_C, N_H, N_V, N_P, N_T)
    packed = host_pack_inputs(x_real, x_imag)
    W1, W2A, W2B = host_consts()

    nc = _get_nc(BC_FULL)
    in_maps = [
        {"xin": packed[i], "w1": W1, "w2a": W2A, "w2b": W2B}
        for i in range(N_CORES)
    ]
    res = run_bass_kernel_spmd(nc, in_maps, core_ids=list(range(N_CORES)),
                               **spmd_kwargs)
    outs = [
        host_unpack_output(r["outp"], BC_FULL).reshape(
            N_B // N_CORES, N_C, HVP, NF)
        for r in res.results
    ]
    return np.concatenate(outs, axis=0), res


def kernel(x_real: np.ndarray, x_imag: np.ndarray) -> np.ndarray:
    return _run(x_real, x_imag)[0]


# revision 17
# speedup vs baseline: 1.3499x; 1.2778x over previous
"""Trainium2 Bass kernel for BeamDelay2AntFreq_3D (nn_BeamDelay2AntFreq_3D).

Math: x [n_b, n_c, h=8, v=4, pol=2, t=48] complex (given as separate
real/imag float32 tensors).
  out[b, c, (h',v',p), f] = sum_{h,v,t} Mh[h',h] Mv[v',v] Mt[f,t] x[b,c,h,v,p,t]
with Mh/Mv = ortho ifft(ifftshift(.)) DFT matrices, Mt = ortho fft(fftshift(.)).
Output complex64 [n_b, n_c, 64, 48].

Sharding: pure data parallel — batch dim (256) split over 8 cores; per core
BC = 1024 independent blocks of [64, 48] complex.

Device pipeline (per 2 blocks):
  Stage 1 (one matmul; fuses the hv-transform with the o<->t transpose):
    lhsT = X tile [128 = (re hvp | im hvp), 96 = (bc2, t)]   f16 data
    rhs  = W1 [128, 128 = (ri, o)]                            f16 const
    psum Y2 [96 = (bc2, t), 128 = (ri, o)]  (fp32 accum)
  Stage 2 (two accumulating matmuls; t-transform, data as stationary):
    lhsT = Y2[:, ri*64:(ri+1)*64] [96, 64 = o] (f16 SBUF copy of psum)
    rhs  = W2A/W2B [96 = (bc2, t), 192 = (bc2, f, ri)]        f16 consts
    psum O [64 = o, 192]; tile_position (0,0)/(0,64) packs two groups
    into 128 partitions.
  PSUM->SBUF copies: ScalarE for stage 1, VectorE for the output.

DMA layouts are host-side reformatted for line-rate DMA (3-6 KB contiguous
runs per partition):
  xin  [128 = (ri, hvp), BC, 48] f16
  outp [128 = (half, o), BC/2, 96] f32, complex-interleaved pairs; host
  unpacks to [BC, 64, 48] complex64.
"""

import numpy as np

import concourse.mybir as mybir
from concourse import bacc
from concourse.tile import TileContext
from concourse.bass_utils import run_bass_kernel_spmd

N_B, N_C, N_H, N_V, N_P, N_T = 256, 32, 8, 4, 2, 48
N_CORES = 8
HVP = N_H * N_V * N_P  # 64
NF = N_T  # 48
BC_FULL = (N_B // N_CORES) * N_C  # 1024 blocks per core

SUP = 64   # bc per DMA super-tile
SUB = 16   # bc per compute sub-tile

W_DT = mybir.dt.float16
W_NP = np.float16


def _transform_mats():
    Mv = np.fft.ifft(np.fft.ifftshift(np.eye(N_V), axes=0), axis=0, norm="ortho")
    Mh = np.fft.ifft(np.fft.ifftshift(np.eye(N_H), axes=0), axis=0, norm="ortho")
    Mt = np.fft.fft(np.fft.fftshift(np.eye(N_T), axes=0), axis=0, norm="ortho")
    M1 = np.kron(np.kron(Mh, Mv), np.eye(N_P))  # [64, 64] complex
    return M1, Mt


def host_consts():
    """W1 [128, 128], W2A/W2B [96, 192] numpy arrays (W_NP dtype)."""
    M1, Mt = _transform_mats()
    M1r, M1i = M1.real, M1.imag
    Mtr, Mti = Mt.real, Mt.imag

    W1 = np.zeros((128, 128), np.float64)
    W1[0:64, 0:64] = M1r.T
    W1[0:64, 64:128] = M1i.T
    W1[64:128, 0:64] = -M1i.T
    W1[64:128, 64:128] = M1r.T

    A_blk = np.zeros((N_T, 2 * NF), np.float64)
    A_blk[:, 0::2] = Mtr.T
    A_blk[:, 1::2] = Mti.T
    B_blk = np.zeros((N_T, 2 * NF), np.float64)
    B_blk[:, 0::2] = -Mti.T
    B_blk[:, 1::2] = Mtr.T
    W2A = np.zeros((96, 192), np.float64)
    W2B = np.zeros((96, 192), np.float64)
    for g in range(2):
        W2A[g * 48:(g + 1) * 48, g * 96:(g + 1) * 96] = A_blk
        W2B[g * 48:(g + 1) * 48, g * 96:(g + 1) * 96] = B_blk
    return W1.astype(W_NP), W2A.astype(W_NP), W2B.astype(W_NP)


def build_nc(bc: int, num_devices: int = N_CORES):
    assert bc % SUP == 0
    nc = bacc.Bacc("TRN2", num_devices=num_devices)

    xin = nc.dram_tensor("xin", [128, bc, N_T], mybir.dt.float16,
                         kind="ExternalInput")
    w1_d = nc.dram_tensor("w1", [128, 128], W_DT, kind="ExternalInput")
    w2a_d = nc.dram_tensor("w2a", [96, 192], W_DT, kind="ExternalInput")
    w2b_d = nc.dram_tensor("w2b", [96, 192], W_DT, kind="ExternalInput")
    outp = nc.dram_tensor("outp", [128, bc // 2, 2 * NF], mybir.dt.float16,
                          kind="ExternalOutput")

    n_sub = SUP // SUB          # 4
    half_pairs = SUP // 4       # 8

    with TileContext(nc) as tc:
        with (
            tc.tile_pool(name="consts", bufs=1) as cpool,
            tc.tile_pool(name="xin_p", bufs=4) as xpool,
            tc.tile_pool(name="y2", bufs=4) as ypool,
            tc.tile_pool(name="outs", bufs=4) as opool,
            tc.tile_pool(name="ps1", bufs=3, space="PSUM") as ps1pool,
            tc.tile_pool(name="ps2", bufs=2, space="PSUM") as ps2pool,
        ):
            w1_t = cpool.tile([128, 128], W_DT, tag="w1")
            w2a_t = cpool.tile([96, 192], W_DT, tag="w2a")
            w2b_t = cpool.tile([96, 192], W_DT, tag="w2b")
            nc.sync.dma_start(w1_t[:, :], w1_d[:, :])
            nc.sync.dma_start(w2a_t[:, :], w2a_d[:, :])
            nc.sync.dma_start(w2b_t[:, :], w2b_d[:, :])

            for it in range(bc // SUP):
                b0 = it * SUP
                x_t = xpool.tile([128, SUP * N_T], mybir.dt.float16, tag="x")
                nc.sync.dma_start(
                    x_t[:, :].rearrange("p (b t) -> p b t", t=N_T),
                    xin[:, b0:b0 + SUP, :],
                )

                o_t = opool.tile([128, (SUP // 2) * 2 * NF],
                                 mybir.dt.float16, tag="o")

                nq = SUB // 4  # 4-bc blocks per sub-tile
                for s in range(n_sub):
                    # sub-tile s covers bc [b0 + s*SUB, b0 + (s+1)*SUB):
                    # 2*nq consecutive pairs starting at pair p0
                    p0 = s * SUB // 2

                    p1 = ps1pool.tile([96, (SUB // 2) * 128],
                                      mybir.dt.float32, tag="p1")
                    for g in range(SUB // 2):
                        pr = p0 + g
                        nc.tensor.matmul(
                            p1[:, g * 128:(g + 1) * 128],
                            lhsT=x_t[:, pr * 96:(pr + 1) * 96],
                            rhs=w1_t[:, :],
                            start=True, stop=True,
                        )
                    # y2 layout (qb, ri, pair, o): stage-2 lhsT slices are
                    # contiguous 128-col blocks. Two permuting copies (one
                    # per ri) keep every AP within 3 free dims.
                    y2 = ypool.tile([96, (SUB // 2) * 128], mybir.dt.float16,
                                    tag="y2")
                    p1v = p1[:, :].rearrange(
                        "k (qb pr ri o) -> k qb pr ri o",
                        qb=SUB // 4, pr=2, ri=2)
                    y2v = y2[:, :].rearrange(
                        "k (qb ri pr o) -> k qb ri pr o",
                        qb=SUB // 4, ri=2, pr=2)
                    for ri in range(2):
                        if s % 2 == 0:
                            nc.scalar.copy(y2v[:, :, ri, :, :],
                                           p1v[:, :, :, ri, :])
                        else:
                            nc.vector.tensor_copy(y2v[:, :, ri, :, :],
                                                  p1v[:, :, :, ri, :])

                    # stage 2: one A/B matmul pair per 4 bc (2 y2 pairs),
                    # lhsT free = (pair, o) -> out partitions (pair, o).
                    # Two 1-bank psum tiles per sub-tile, 2 q-blocks each.
                    for h in range(nq // 2):
                        p2 = ps2pool.tile([128, 384], mybir.dt.float32,
                                          tag="p2")
                        for qq in range(2):
                            q = 2 * h + qq
                            c0 = qq * 192
                            lhsA = y2[:, q * 256:q * 256 + 128]
                            lhsB = y2[:, q * 256 + 128:q * 256 + 256]
                            nc.tensor.matmul(
                                p2[:, c0:c0 + 192], lhsT=lhsA,
                                rhs=w2a_t[:, :], start=True, stop=False,
                            )
                            nc.tensor.matmul(
                                p2[:, c0:c0 + 192], lhsT=lhsB,
                                rhs=w2b_t[:, :], start=False, stop=True,
                            )
                        o_slice = o_t[:, (s * nq + 2 * h) * 192:
                                      (s * nq + 2 * h + 2) * 192]
                        if (s + h) % 2 == 0:
                            nc.vector.tensor_copy(o_slice, p2[:, :])
                        else:
                            nc.scalar.copy(o_slice, p2[:, :])

                nc.sync.dma_start(
                    outp[:, it * (SUP // 2):(it + 1) * (SUP // 2), :],
                    o_t[:, :].rearrange("p (b c) -> p b c", c=2 * NF),
                )
    return nc


def host_pack_inputs(x_real, x_imag):
    """f32 [256, 32, 8, 4, 2, 48] pair -> [cores, 128, BC, 48] f16."""
    xr = np.asarray(x_real, np.float16).reshape(N_CORES, BC_FULL, HVP, N_T)
    xi = np.asarray(x_imag, np.float16).reshape(N_CORES, BC_FULL, HVP, N_T)
    packed = np.empty((N_CORES, 128, BC_FULL, N_T), np.float16)
    packed[:, :64] = xr.transpose(0, 2, 1, 3)
    packed[:, 64:] = xi.transpose(0, 2, 1, 3)
    return packed


def host_unpack_output(outp, bc):
    """outp [128, bc/2, 96] f16 -> [bc, 64, 48] complex64.

    outp row = half*64 + o; col-slot = qg*2 + g; bc = qg*4 + half*2 + g."""
    v = outp.astype(np.float32).view(np.complex64).reshape(
        2, 64, bc // 4, 2, NF)
    # [half, o, qg, g, f] -> [qg, half, g, o, f]
    return np.ascontiguousarray(v.transpose(2, 0, 3, 1, 4)).reshape(
        bc, HVP, NF)


def reference_numpy(x_real, x_imag):
    """Pure-numpy reference; x_* [bc, 64, 48] -> [bc, 64, 48] complex64."""
    M1, Mt = _transform_mats()
    x = x_real.astype(np.complex64) + 1j * x_imag.astype(np.complex64)
    y = np.einsum("oi,bit->bot", M1, x)
    return np.einsum("ft,bot->bof", Mt, y).astype(np.complex64)


_NC_CACHE = {}


def _get_nc(bc):
    if bc not in _NC_CACHE:
        nc = build_nc(bc)
        nc.compile()
        _NC_CACHE[bc] = nc
    return _NC_CACHE[bc]


def _run(x_real: np.ndarray, x_imag: np.ndarray, **spmd_kwargs):
    assert x_real.shape == (N_B, N_C, N_H, N_V, N_P, N_T)
    packed = host_pack_inputs(x_real, x_imag)
    W1, W2A, W2B = host_consts()

    nc = _get_nc(BC_FULL)
    in_maps = [
        {"xin": packed[i], "w1": W1, "w2a": W2A, "w2b": W2B}
        for i in range(N_CORES)
    ]
    res = run_bass_kernel_spmd(nc, in_maps, core_ids=list(range(N_CORES)),
                               **spmd_kwargs)
    outs = [
        host_unpack_output(r["outp"], BC_FULL).reshape(
            N_B // N_CORES, N_C, HVP, NF)
        for r in res.results
    ]
    return np.concatenate(outs, axis=0), res


def kernel(x_real: np.ndarray, x_imag: np.ndarray) -> np.ndarray:
    return _run(x_real, x_imag)[0]


# revision 18
# speedup vs baseline: 1.3502x; 1.0002x over previous
"""Trainium2 Bass kernel for BeamDelay2AntFreq_3D (nn_BeamDelay2AntFreq_3D).

Math: x [n_b, n_c, h=8, v=4, pol=2, t=48] complex (given as separate
real/imag float32 tensors).
  out[b, c, (h',v',p), f] = sum_{h,v,t} Mh[h',h] Mv[v',v] Mt[f,t] x[b,c,h,v,p,t]
with Mh/Mv = ortho ifft(ifftshift(.)) DFT matrices, Mt = ortho fft(fftshift(.)).
Output complex64 [n_b, n_c, 64, 48].

Sharding: pure data parallel — batch dim (256) split over 8 cores; per core
BC = 1024 independent blocks of [64, 48] complex.

Device pipeline (per 2 blocks):
  Stage 1 (one matmul; fuses the hv-transform with the o<->t transpose):
    lhsT = X tile [128 = (re hvp | im hvp), 96 = (bc2, t)]   f16 data
    rhs  = W1 [128, 128 = (ri, o)]                            f16 const
    psum Y2 [96 = (bc2, t), 128 = (ri, o)]  (fp32 accum)
  Stage 2 (two accumulating matmuls; t-transform, data as stationary):
    lhsT = Y2[:, ri*64:(ri+1)*64] [96, 64 = o] (f16 SBUF copy of psum)
    rhs  = W2A/W2B [96 = (bc2, t), 192 = (bc2, f, ri)]        f16 consts
    psum O [64 = o, 192]; tile_position (0,0)/(0,64) packs two groups
    into 128 partitions.
  PSUM->SBUF copies: ScalarE for stage 1, VectorE for the output.

DMA layouts are host-side reformatted for line-rate DMA (3-6 KB contiguous
runs per partition):
  xin  [128 = (ri, hvp), BC, 48] f16
  outp [128 = (half, o), BC/2, 96] f32, complex-interleaved pairs; host
  unpacks to [BC, 64, 48] complex64.
"""

import numpy as np

import concourse.mybir as mybir
from concourse import bacc
from concourse.tile import TileContext
from concourse.bass_utils import run_bass_kernel_spmd

N_B, N_C, N_H, N_V, N_P, N_T = 256, 32, 8, 4, 2, 48
N_CORES = 8
HVP = N_H * N_V * N_P  # 64
NF = N_T  # 48
BC_FULL = (N_B // N_CORES) * N_C  # 1024 blocks per core

SUP = 64   # bc per DMA super-tile
SUB = 16   # bc per compute sub-tile

W_DT = mybir.dt.float16
W_NP = np.float16


def _transform_mats():
    Mv = np.fft.ifft(np.fft.ifftshift(np.eye(N_V), axes=0), axis=0, norm="ortho")
    Mh = np.fft.ifft(np.fft.ifftshift(np.eye(N_H), axes=0), axis=0, norm="ortho")
    Mt = np.fft.fft(np.fft.fftshift(np.eye(N_T), axes=0), axis=0, norm="ortho")
    M1 = np.kron(np.kron(Mh, Mv), np.eye(N_P))  # [64, 64] complex
    return M1, Mt


def host_consts():
    """W1 [128, 128], W2A/W2B [96, 192] numpy arrays (W_NP dtype)."""
    M1, Mt = _transform_mats()
    M1r, M1i = M1.real, M1.imag
    Mtr, Mti = Mt.real, Mt.imag

    W1 = np.zeros((128, 128), np.float64)
    W1[0:64, 0:64] = M1r.T
    W1[0:64, 64:128] = M1i.T
    W1[64:128, 0:64] = -M1i.T
    W1[64:128, 64:128] = M1r.T

    A_blk = np.zeros((N_T, 2 * NF), np.float64)
    A_blk[:, 0::2] = Mtr.T
    A_blk[:, 1::2] = Mti.T
    B_blk = np.zeros((N_T, 2 * NF), np.float64)
    B_blk[:, 0::2] = -Mti.T
    B_blk[:, 1::2] = Mtr.T
    W2A = np.zeros((96, 192), np.float64)
    W2B = np.zeros((96, 192), np.float64)
    for g in range(2):
        W2A[g * 48:(g + 1) * 48, g * 96:(g + 1) * 96] = A_blk
        W2B[g * 48:(g + 1) * 48, g * 96:(g + 1) * 96] = B_blk
    return W1.astype(W_NP), W2A.astype(W_NP), W2B.astype(W_NP)


def build_nc(bc: int, num_devices: int = N_CORES):
    assert bc % SUP == 0
    nc = bacc.Bacc("TRN2", num_devices=num_devices)

    xin = nc.dram_tensor("xin", [128, bc, N_T], mybir.dt.float16,
                         kind="ExternalInput")
    w1_d = nc.dram_tensor("w1", [128, 128], W_DT, kind="ExternalInput")
    w2a_d = nc.dram_tensor("w2a", [96, 192], W_DT, kind="ExternalInput")
    w2b_d = nc.dram_tensor("w2b", [96, 192], W_DT, kind="ExternalInput")
    outp = nc.dram_tensor("outp", [128, bc // 2, 2 * NF], mybir.dt.float16,
                          kind="ExternalOutput")

    n_sub = SUP // SUB          # 4
    half_pairs = SUP // 4       # 8

    with TileContext(nc) as tc:
        with (
            tc.tile_pool(name="consts", bufs=1) as cpool,
            tc.tile_pool(name="xin_p", bufs=6) as xpool,
            tc.tile_pool(name="y2", bufs=6) as ypool,
            tc.tile_pool(name="outs", bufs=6) as opool,
            tc.tile_pool(name="ps1", bufs=3, space="PSUM") as ps1pool,
            tc.tile_pool(name="ps2", bufs=2, space="PSUM") as ps2pool,
        ):
            w1_t = cpool.tile([128, 128], W_DT, tag="w1")
            w2a_t = cpool.tile([96, 192], W_DT, tag="w2a")
            w2b_t = cpool.tile([96, 192], W_DT, tag="w2b")
            nc.sync.dma_start(w1_t[:, :], w1_d[:, :])
            nc.sync.dma_start(w2a_t[:, :], w2a_d[:, :])
            nc.sync.dma_start(w2b_t[:, :], w2b_d[:, :])

            for it in range(bc // SUP):
                b0 = it * SUP
                x_t = xpool.tile([128, SUP * N_T], mybir.dt.float16, tag="x")
                nc.sync.dma_start(
                    x_t[:, :].rearrange("p (b t) -> p b t", t=N_T),
                    xin[:, b0:b0 + SUP, :],
                )

                o_t = opool.tile([128, (SUP // 2) * 2 * NF],
                                 mybir.dt.float16, tag="o")

                nq = SUB // 4  # 4-bc blocks per sub-tile
                for s in range(n_sub):
                    # sub-tile s covers bc [b0 + s*SUB, b0 + (s+1)*SUB):
                    # 2*nq consecutive pairs starting at pair p0
                    p0 = s * SUB // 2

                    p1 = ps1pool.tile([96, (SUB // 2) * 128],
                                      mybir.dt.float32, tag="p1")
                    for g in range(SUB // 2):
                        pr = p0 + g
                        nc.tensor.matmul(
                            p1[:, g * 128:(g + 1) * 128],
                            lhsT=x_t[:, pr * 96:(pr + 1) * 96],
                            rhs=w1_t[:, :],
                            start=True, stop=True,
                        )
                    # y2 layout (qb, ri, pair, o): stage-2 lhsT slices are
                    # contiguous 128-col blocks. Two permuting copies (one
                    # per ri) keep every AP within 3 free dims.
                    y2 = ypool.tile([96, (SUB // 2) * 128], mybir.dt.float16,
                                    tag="y2")
                    p1v = p1[:, :].rearrange(
                        "k (qb pr ri o) -> k qb pr ri o",
                        qb=SUB // 4, pr=2, ri=2)
                    y2v = y2[:, :].rearrange(
                        "k (qb ri pr o) -> k qb ri pr o",
                        qb=SUB // 4, ri=2, pr=2)
                    for ri in range(2):
                        if s % 2 == 0:
                            nc.scalar.copy(y2v[:, :, ri, :, :],
                                           p1v[:, :, :, ri, :])
                        else:
                            nc.vector.tensor_copy(y2v[:, :, ri, :, :],
                                                  p1v[:, :, :, ri, :])

                    # stage 2: one A/B matmul pair per 4 bc (2 y2 pairs),
                    # lhsT free = (pair, o) -> out partitions (pair, o).
                    # Two 1-bank psum tiles per sub-tile, 2 q-blocks each.
                    for h in range(nq // 2):
                        p2 = ps2pool.tile([128, 384], mybir.dt.float32,
                                          tag="p2")
                        for qq in range(2):
                            q = 2 * h + qq
                            c0 = qq * 192
                            lhsA = y2[:, q * 256:q * 256 + 128]
                            lhsB = y2[:, q * 256 + 128:q * 256 + 256]
                            nc.tensor.matmul(
                                p2[:, c0:c0 + 192], lhsT=lhsA,
                                rhs=w2a_t[:, :], start=True, stop=False,
                            )
                            nc.tensor.matmul(
                                p2[:, c0:c0 + 192], lhsT=lhsB,
                                rhs=w2b_t[:, :], start=False, stop=True,
                            )
                        o_slice = o_t[:, (s * nq + 2 * h) * 192:
                                      (s * nq + 2 * h + 2) * 192]
                        if (s + h) % 2 == 0:
                            nc.vector.tensor_copy(o_slice, p2[:, :])
                        else:
                            nc.scalar.copy(o_slice, p2[:, :])

                nc.sync.dma_start(
                    outp[:, it * (SUP // 2):(it + 1) * (SUP // 2), :],
                    o_t[:, :].rearrange("p (b c) -> p b c", c=2 * NF),
                )
    return nc


def host_pack_inputs(x_real, x_imag):
    """f32 [256, 32, 8, 4, 2, 48] pair -> [cores, 128, BC, 48] f16."""
    xr = np.asarray(x_real, np.float16).reshape(N_CORES, BC_FULL, HVP, N_T)
    xi = np.asarray(x_imag, np.float16).reshape(N_CORES, BC_FULL, HVP, N_T)
    packed = np.empty((N_CORES, 128, BC_FULL, N_T), np.float16)
    packed[:, :64] = xr.transpose(0, 2, 1, 3)
    packed[:, 64:] = xi.transpose(0, 2, 1, 3)
    return packed


def host_unpack_output(outp, bc):
    """outp [128, bc/2, 96] f16 -> [bc, 64, 48] complex64.

    outp row = half*64 + o; col-slot = qg*2 + g; bc = qg*4 + half*2 + g."""
    v = outp.astype(np.float32).view(np.complex64).reshape(
        2, 64, bc // 4, 2, NF)
    # [half, o, qg, g, f] -> [qg, half, g, o, f]
    return np.ascontiguousarray(v.transpose(2, 0, 3, 1, 4)).reshape(
        bc, HVP, NF)


def reference_numpy(x_real, x_imag):
    """Pure-numpy reference; x_* [bc, 64, 48] -> [bc, 64, 48] complex64."""
    M1, Mt = _transform_mats()
    x = x_real.astype(np.complex64) + 1j * x_imag.astype(np.complex64)
    y = np.einsum("oi,bit->bot", M1, x)
    return np.einsum("ft,bot->bof", Mt, y).astype(np.complex64)


_NC_CACHE = {}


def _get_nc(bc):
    if bc not in _NC_CACHE:
        nc = build_nc(bc)
        nc.compile()
        _NC_CACHE[bc] = nc
    return _NC_CACHE[bc]


def _run(x_real: np.ndarray, x_imag: np.ndarray, **spmd_kwargs):
    assert x_real.shape == (N_B, N_C, N_H, N_V, N_P, N_T)
    packed = host_pack_inputs(x_real, x_imag)
    W1, W2A, W2B = host_consts()

    nc = _get_nc(BC_FULL)
    in_maps = [
        {"xin": packed[i], "w1": W1, "w2a": W2A, "w2b": W2B}
        for i in range(N_CORES)
    ]
    res = run_bass_kernel_spmd(nc, in_maps, core_ids=list(range(N_CORES)),
                               **spmd_kwargs)
    outs = [
        host_unpack_output(r["outp"], BC_FULL).reshape(
            N_B // N_CORES, N_C, HVP, NF)
        for r in res.results
    ]
    return np.concatenate(outs, axis=0), res


def kernel(x_real: np.ndarray, x_imag: np.ndarray) -> np.ndarray:
    return _run(x_real, x_imag)[0]
